# revision 21
# baseline (speedup 1.0000x reference)
"""Trainium2 Bass kernel for BatchedLinearHolomorphicEQProp (8 NeuronCores).

Math: the reference settles  s <- [x, tanh(s[IN:] - LR * (s @ Wsym)[IN:])]
for T steps.  Since the input slice is clamped, this reduces to iterating

    r <- tanh([r, x] @ M_big),   M_big = [[I - LR*Wsym[IN:, IN:]],
                                          [   -LR*Wsym[:IN, IN:]]]

Kernel design (tensor parallel over 8 cores on one TRN2 chip):
  - M_big (bf16) is column-sharded: 1152 columns per core, SBUF-resident
    for all 500 steps (the only way to beat streaming 340MB/step from HBM).
  - The state r is kept TRANSPOSED and replicated: 72 k-tiles [128,16]
    (+ 4 constant x^T tiles).  Each step, each core:
      * 76 k-tiles x 4 column-group matmuls (tile_position col tiling:
        batch=16 occupies 4 x 16 PE columns), accumulating [16@32g, 288]
        in PSUM fp32;
      * ACT tanh -> bf16;  PE transposes (32-col blocks) back to k-tile
        layout;  DVE copy to a send buffer;
      * gpsimd remote_dma_broadcast sends its 9 tiles to all 8 cores'
        replicated state buffer (slot = own core id, double-buffered by
        step parity); monotonic semaphores with register thresholds
        synchronize across cores - no ncfw collectives, no loop barriers.
"""
from contextlib import ExitStack

import numpy as np
import ml_dtypes

IN_SZ = 512
TOTAL = 9216
REST = TOTAL - IN_SZ          # 8704
B = 16
LR = 0.5
N_CORES = 8
TPC = 9                        # state k-tiles per core
NX = 4                         # x k-tiles (4*128 = 512 = IN_SZ)
C = 128 * TPC                  # 1152 columns per core
GN = 32 * TPC                  # 288 columns per col group
NS = N_CORES * TPC             # 72 state k-tiles
KT = NS + NX                   # 76 contraction k-tiles
SL = 16 * TPC                  # 144 free elems per core slot (transposed)
RP = N_CORES * C               # 9216 padded rest size
NCH = 3                        # exchange chunks per step (pipelined sends)


def _ch_splits():
    """Split TPC tiles into NCH chunk boundaries, e.g. 9 -> [0,3,6,9]."""
    base, rem = divmod(TPC, NCH)
    sizes = [base + (1 if h < rem else 0) for h in range(NCH)]
    lo = [sum(sizes[:h]) for h in range(NCH)]
    return lo + [TPC]

NP_HALF = ml_dtypes.bfloat16


def _build_nc(T: int):
    import concourse.bass as bass
    import concourse.mybir as mybir
    from concourse import bacc

    F32 = mybir.dt.float32
    dt_half = mybir.dt.bfloat16
    assert T % 2 == 0 and T >= 4
    iters = T // 2

    nc = bacc.Bacc("TRN2", target_bir_lowering=False, debug=False,
                   num_devices=N_CORES)

    m_in = nc.dram_tensor("m_in", [128, KT * C], dt_half, kind="ExternalInput")
    xt_in = nc.dram_tensor("xt_in", [128, NX * 16], dt_half,
                           kind="ExternalInput")
    eye_in = nc.dram_tensor("eye_in", [128, 16], dt_half, kind="ExternalInput")
    out_d = nc.dram_tensor("out", [128, GN], F32, kind="ExternalOutput")

    m_sb = nc.alloc_sbuf_tensor("m_sb", [128, KT * C], dt_half)
    xt_sb = nc.alloc_sbuf_tensor("xt_sb", [128, NX * 16], dt_half)
    eye_sb = nc.alloc_sbuf_tensor("eye_sb", [128, 16], dt_half)
    recv_sb = nc.alloc_sbuf_tensor("recv_sb", [128, 2 * NS * 16], dt_half)
    send_sb = nc.alloc_sbuf_tensor("send_sb", [128, 2 * SL], dt_half)
    st_sb = nc.alloc_sbuf_tensor("st_sb", [128, 2 * GN], dt_half)
    out_sb = nc.alloc_sbuf_tensor("out_sb", [128, GN], F32)

    ps = [nc.alloc_psum_tensor(f"ps{k}", [128, GN], F32) for k in range(2)]
    pst = [nc.alloc_psum_tensor(f"pst{k}", [128, SL], dt_half)
           for k in range(2)]
    ps_scratch = nc.alloc_psum_tensor("ps_scratch", [128, GN], F32)

    ctx = ExitStack()
    nc._sem_ctx = ctx
    sem = lambda name: ctx.enter_context(nc.semaphore(name))
    s_load = sem("s_load")
    s_init = sem("s_init")
    # chunked exchange: chunk h = tiles [CH[h], CH[h+1])
    s_arr = [[sem(f"s_arr{h}_{k}") for k in range(2)] for h in range(NCH)]
    s_mm = [sem(f"s_mm{k}") for k in range(2)]
    s_tanh = [[sem(f"s_tanh{h}_{k}") for k in range(2)] for h in range(NCH)]
    s_tp = [[sem(f"s_tp{h}_{k}") for k in range(2)] for h in range(NCH)]
    s_cp = [[sem(f"s_cp{h}_{k}") for k in range(2)] for h in range(NCH)]
    s_prep = sem("s_prep")
    s_loc = sem("s_loc")
    s_out = sem("s_out")
    s_bar = sem("s_bar")

    n_dma_loads = KT + 2
    CH = _ch_splits()
    CH_LO = CH[:-1]
    CH_HI = CH[1:]
    CH_N = [CH_HI[h] - CH_LO[h] for h in range(NCH)]
    # k-loop consumption order: chunk-major (all cores' chunk-0 tiles first)
    K_ORDER = [(c, b) for h in range(NCH)
               for c in range(N_CORES) for b in range(CH_LO[h], CH_HI[h])]
    J0 = [N_CORES * CH_LO[h] for h in range(NCH)]

    def recv_ap(par, c, b):
        off = par * NS * 16 + c * SL + b * 16
        return recv_sb[:, off:off + 16]

    with nc.Block() as block:

        @block.sync
        def _(sync):
            for t in range(KT):
                sync.dma_start(m_sb[:, t * C:(t + 1) * C],
                               m_in[:, t * C:(t + 1) * C]).then_inc(s_load, 16)
            sync.dma_start(xt_sb[:, :], xt_in[:, :]).then_inc(s_load, 16)
            sync.dma_start(eye_sb[:, :], eye_in[:, :]).then_inc(s_load, 16)
            sync.wait_ge(s_out, 1)
            for h in range(NCH):
                for k in range(2):
                    sync.wait_ge(s_arr[h][k], 16 * (T // 2))
            sync.dma_start(out_d[:, :], out_sb[:, :]).then_inc(s_load, 16)
            sync.wait_ge(s_load, 16 * (n_dma_loads + 1))

        @block.vector
        def _(vector):
            vector.memset(recv_sb[:, :], 0).then_inc(s_init, 1)
            for k in range(2):
                vector.memset(ps[k][:, :], 0).then_inc(s_init, 1)
            with vector.Fori(0, iters) as i:
                for k in range(2):
                    # sends of steps <= s-1 drained (s_loc: +16 barrier,
                    # +16*NCH/step), then copy each chunk as it is transposed
                    vector.wait_ge(s_loc, ((i * 2 + k) * NCH + 1) * 16)
                    for h in range(NCH):
                        vector.wait_ge(s_tp[h][k], i + 1)
                        lo, hi = 16 * CH_LO[h], 16 * CH_HI[h]
                        vector.tensor_copy(
                            send_sb[:, k * SL + lo: k * SL + hi],
                            pst[k][:, lo:hi]).then_inc(s_cp[h][k], 1)

        @block.tensor
        def _(tensor):
            tensor.wait_ge(s_load, 16 * n_dma_loads)
            tensor.wait_ge(s_init, 3)

            N_DUMMY = 24

            def mm4(t, w, start, stop):
                mm = None
                for g in range(4):
                    mm = tensor.matmul(
                        ps[k_cur][32 * g:32 * g + B, :],
                        w,
                        m_sb[:, t * C + g * GN: t * C + (g + 1) * GN],
                        start=start, stop=stop,
                        tile_position=(0, 32 * g),
                        skip_group_check=True,
                    )
                return mm

            def step(i, k):
                nonlocal k_cur
                k_cur = k
                # constant x-tile matmuls first: real PE work that does not
                # depend on the exchange (keeps HAM warm during the wait)
                tensor.wait_ge(s_tanh[NCH - 1][k], i)  # psum WAR, tanh of s-2
                for t in range(NS, KT):
                    mm4(t, xt_sb[:, (t - NS) * 16:(t - NS) * 16 + 16],
                        t == NS, False)
                # chunk-0 tiles of every core arrive first; consume them
                # while later chunks' broadcasts are still in flight
                mm = None
                for h in range(NCH):
                    tensor.wait_ge(s_arr[h][k], (i + k) * 16)
                    for j in range(J0[h], J0[h] + N_CORES * CH_N[h]):
                        c, b = K_ORDER[j]
                        mm = mm4(j, recv_ap(k, c, b), False, j == NS - 1)
                mm.then_inc(s_mm[k], 1)
                # transposes + sends per chunk: early chunks' broadcasts
                # launch while later chunks are still being transposed
                for h in range(NCH):
                    tensor.wait_ge(s_tanh[h][k], i + 1)
                    tensor.wait_ge(s_cp[h][k], i)
                    tp = None
                    for b in range(CH_LO[h], CH_HI[h]):
                        for gi in range(4):
                            tp = tensor.transpose(
                                pst[k][32 * gi:32 * gi + 32,
                                       b * 16:(b + 1) * 16],
                                st_sb[32 * gi:32 * gi + B,
                                      k * GN + b * 32: k * GN + (b + 1) * 32],
                                eye_sb[32 * gi:32 * gi + B, :],
                                tile_position=(32 * gi, 32 * gi),
                            )
                    tp.then_inc(s_tp[h][k], 1)
                # dummy matmuls into a scratch bank: keep the PE array busy
                # while the broadcasts are in flight so HAM stays warm
                for dmy in range(N_DUMMY):
                    for g in range(4):
                        tensor.matmul(
                            ps_scratch[32 * g:32 * g + B, :],
                            xt_sb[:, 0:16],
                            m_sb[:, g * GN:(g + 1) * GN],
                            start=True, stop=True,
                            tile_position=(0, 32 * g),
                            skip_group_check=True,
                        )

            k_cur = 0
            with tensor.Fori(0, iters) as i:
                step(i, 0)
                step(i, 1)

        @block.scalar
        def _(scalar):
            import concourse.mybir as mybir
            with scalar.Fori(0, iters) as i:
                for k in range(2):
                    scalar.wait_ge(s_mm[k], i + 1)
                    for h in range(NCH):
                        scalar.wait_ge(s_tp[h][k], i)   # WAR on st_sb
                        lo, hi = 32 * CH_LO[h], 32 * CH_HI[h]
                        scalar.activation(
                            st_sb[:, k * GN + lo: k * GN + hi],
                            ps[k][:, lo:hi],
                            mybir.ActivationFunctionType.Tanh,
                        ).then_inc(s_tanh[h][k], 1)
            kf = (T - 1) % 2
            scalar.wait_ge(s_mm[kf], iters)
            scalar.activation(out_sb[:, :], ps[kf][:, :],
                              mybir.ActivationFunctionType.Tanh,
                              ).then_inc(s_out, 1)

        @block.gpsimd
        def _(gpsimd):
            pid = gpsimd.partition_id()
            rdests = [(0, kk) for kk in range(N_CORES)]
            gpsimd.wait_ge(s_init, 3)
            gpsimd.remote_sem_update_broadcast(
                s_bar, s_loc, rdests=rdests).then_inc(s_prep, 1)
            gpsimd.wait_ge(s_prep, 1)
            gpsimd.trigger_dma(1)
            gpsimd.wait_ge(s_bar, 16)
            with gpsimd.Fori(0, iters) as i:
                for k in range(2):
                    dst_par = 1 - k
                    for h in range(NCH):
                        lo, hi = 16 * CH_LO[h], 16 * CH_HI[h]
                        out_ap = recv_sb[:, bass.ds(
                            pid * SL + dst_par * NS * 16 + lo, hi - lo)]
                        gpsimd.remote_dma_broadcast(
                            out_ap, send_sb[:, k * SL + lo: k * SL + hi],
                            remote_sem=s_arr[h][dst_par], local_sem=s_loc,
                            rdests=rdests,
                        ).then_inc(s_prep, 1)
                        gpsimd.wait_ge(s_cp[h][k], i + 1)
                        gpsimd.wait_ge(s_prep, (i * 2 + k) * NCH + h + 2)
                        gpsimd.trigger_dma(1)
            gpsimd.wait_ge(s_loc, 16 * (NCH * T + 1))
            for h in range(NCH):
                for k in range(2):
                    gpsimd.wait_ge(s_arr[h][k], 16 * (T // 2))

    nc.compile()
    return nc


def _prep_inputs(x: np.ndarray, W: np.ndarray):
    Wsym = 0.5 * (W.astype(np.float32) + W.astype(np.float32).T)
    M_big = np.zeros((KT * 128, RP), dtype=np.float32)
    M_big[:REST, :REST] = -LR * Wsym[IN_SZ:, IN_SZ:]
    M_big[:REST, :REST] += np.eye(REST, dtype=np.float32)
    M_big[RP:RP + IN_SZ, :REST] = -LR * Wsym[:IN_SZ, IN_SZ:]

    perm = np.empty(C, dtype=np.int64)
    p = 0
    for g in range(4):
        for j in range(TPC):
            perm[p:p + 32] = 128 * j + 32 * g + np.arange(32)
            p += 32

    xt = np.zeros((128, NX * 16), dtype=np.float32)
    for u in range(NX):
        xt[:, u * 16:u * 16 + B] = x[:, 128 * u:128 * (u + 1)].T
    eye = np.zeros((128, 16), dtype=np.float32)
    for gi in range(4):
        eye[32 * gi + np.arange(16), np.arange(16)] = 1.0

    xt_h = np.ascontiguousarray(xt.astype(NP_HALF))
    eye_h = np.ascontiguousarray(eye.astype(NP_HALF))
    # k-loop block order (must match K_ORDER in _build_nc): chunk-major
    CH = _ch_splits()
    order = ([TPC * cc + b for h in range(NCH)
              for cc in range(N_CORES) for b in range(CH[h], CH[h + 1])]
             + list(range(NS, KT)))          # x blocks stay at the end
    in_maps = []
    for c in range(N_CORES):
        shard = M_big[:, c * C:(c + 1) * C][:, perm]
        m_loc = shard.reshape(KT, 128, C)[order]
        m_loc = m_loc.transpose(1, 0, 2).reshape(128, -1)
        in_maps.append({
            "m_in": np.ascontiguousarray(m_loc.astype(NP_HALF)),
            "xt_in": xt_h,
            "eye_in": eye_h,
        })
    return in_maps


def _extract_output(out_core: np.ndarray):
    """out_core: [128, GN] fp32 from core 7; output = global cols 8192:8704
    = core 7 local cols 128:640 (pre-permutation)."""
    res = np.empty((B, 512), dtype=np.float32)
    for cl in range(128, 640):
        m = cl // 32
        res[:, cl - 128] = out_core[32 * (m % 4):32 * (m % 4) + B,
                                    32 * (m // 4) + (cl % 32)]
    return res


_CACHE = {}


def _make_exec(T):
    """Compile the NEFF and build a jitted shard_map executor whose inputs
    are device-resident jax Arrays — avoids the ~180MB host->device upload
    landing inside the NEFF execution window (which skews core start times
    by several ms and stalls the first cross-core exchange)."""
    import jax
    import numpy as np
    from jax.experimental.shard_map import shard_map
    from jax.sharding import Mesh, PartitionSpec
    import concourse.mybir as mybir
    from concourse import bass2jax

    bass2jax.install_neuronx_cc_hook()
    nc = _build_nc(T)

    partition_name = (nc.partition_id_tensor.name
                      if nc.partition_id_tensor else None)
    in_names, out_names, out_avals, zero_shapes = [], [], [], []
    for alloc in nc.m.functions[0].allocations:
        if not isinstance(alloc, mybir.MemoryLocationSet):
            continue
        name = alloc.memorylocations[0].name
        if alloc.kind == "ExternalInput":
            if name != partition_name:
                in_names.append(name)
        elif alloc.kind == "ExternalOutput":
            shape = tuple(alloc.tensor_shape)
            dtype = mybir.dt.np(alloc.dtype)
            out_names.append(name)
            out_avals.append(jax.core.ShapedArray(shape, dtype))
            zero_shapes.append((shape, dtype))
    n_params = len(in_names)
    all_names = list(in_names) + list(out_names)
    if partition_name is not None:
        all_names.append(partition_name)
    donate = tuple(range(n_params, n_params + len(out_names)))

    perdev = bool(int(__import__("os").environ.get("KERNEL_PERDEV", "0")))

    if perdev:
        # 8 independent single-device executions dispatched asynchronously,
        # with the partition id fed as an explicit input (PartitionIdOp only
        # yields the right rank under shard_map).
        def _body(*args):
            outs = bass2jax._bass_exec_p.bind(
                *args,
                out_avals=tuple(out_avals),
                in_names=tuple(all_names),
                out_names=tuple(out_names),
                lowering_input_output_aliases=(),
                sim_require_finite=True,
                sim_require_nnan=True,
                nc=nc,
            )
            return tuple(outs)

        devices = jax.devices()[:N_CORES]
        fn = jax.jit(_body, donate_argnums=donate, keep_unused=True)
        return dict(nc=nc, fn=fn, in_names=in_names, out_names=out_names,
                    zero_shapes=zero_shapes, devices=devices,
                    partition_name=partition_name, perdev=True)

    def _body(*args):
        operands = list(args)
        if partition_name is not None:
            operands.append(bass2jax.partition_id_tensor())
        outs = bass2jax._bass_exec_p.bind(
            *operands,
            out_avals=tuple(out_avals),
            in_names=tuple(all_names),
            out_names=tuple(out_names),
            lowering_input_output_aliases=(),
            sim_require_finite=True,
            sim_require_nnan=True,
            nc=nc,
        )
        return tuple(outs)

    devices = jax.devices()[:N_CORES]
    mesh = Mesh(np.asarray(devices), ("core",))
    nspecs = n_params + len(out_names)
    sharded = jax.jit(
        shard_map(_body, mesh=mesh,
                  in_specs=(PartitionSpec("core"),) * nspecs,
                  out_specs=(PartitionSpec("core"),) * len(out_names),
                  check_rep=False),
        donate_argnums=donate, keep_unused=True,
    )
    shard = jax.sharding.NamedSharding(mesh, PartitionSpec("core"))
    return dict(nc=nc, fn=sharded, in_names=in_names, out_names=out_names,
                zero_shapes=zero_shapes, shard=shard, perdev=False)


def _run(x, W, T, trace=False):
    """Returns {core_id: {out_name: np.ndarray}} like BassKernelResults.results
    plus None exec time (use test.py for profiling)."""
    import jax
    if T not in _CACHE:
        _CACHE[T] = _make_exec(T)
    ex = _CACHE[T]
    in_maps = _prep_inputs(x, W)
    import numpy as np

    class _Res:
        pass

    res = _Res()
    res.exec_time_ns = None

    if ex.get("perdev"):
        per_core_outs = _run_perdev(ex, in_maps)
        res.results = per_core_outs
        return res

    placed = [
        jax.device_put(
            np.concatenate([in_maps[c][name] for c in range(N_CORES)], axis=0),
            ex["shard"])
        for name in ex["in_names"]
    ]
    zeros = [
        jax.device_put(np.zeros((N_CORES * s[0], *s[1:]), dt), ex["shard"])
        for (s, dt) in ex["zero_shapes"]
    ]
    for a in placed + zeros:
        a.block_until_ready()
    outs = ex["fn"](*placed, *zeros)
    outs = [np.asarray(o) for o in outs]
    res.results = [
        {name: outs[i].reshape(N_CORES, -1, *outs[i].shape[1:])[c]
         for i, name in enumerate(ex["out_names"])}
        for c in range(N_CORES)
    ]
    return res


def _run_perdev(ex, in_maps):
    import jax
    import numpy as np
    devices = ex["devices"]
    all_args = []
    for c in range(N_CORES):
        args = [jax.device_put(in_maps[c][n], devices[c])
                for n in ex["in_names"]]
        args += [jax.device_put(np.zeros(s, dt), devices[c])
                 for (s, dt) in ex["zero_shapes"]]
        if ex["partition_name"] is not None:
            args.append(jax.device_put(np.array([[c]], np.uint32),
                                       devices[c]))
        all_args.append(args)
    for args in all_args:
        for a in args:
            a.block_until_ready()
    futs = [ex["fn"](*all_args[c]) for c in range(N_CORES)]  # async dispatch
    outs = [[np.asarray(o) for o in f] for f in futs]
    return [
        {name: outs[c][i] for i, name in enumerate(ex["out_names"])}
        for c in range(N_CORES)
    ]


def kernel(x: np.ndarray, W: np.ndarray, T_settle) -> np.ndarray:
    T = int(np.asarray(T_settle))
    res = _run(np.asarray(x, np.float32), np.asarray(W, np.float32), T)
    return _extract_output(res.results[7]["out"])


# revision 22
# speedup vs baseline: 1.3556x; 1.3556x over previous
"""Trainium2 Bass kernel for BatchedLinearHolomorphicEQProp (8 NeuronCores).

Math: the reference settles  s <- [x, tanh(s[IN:] - LR * (s @ Wsym)[IN:])]
for T steps.  Since the input slice is clamped, this reduces to iterating

    r <- tanh([r, x] @ M_big),   M_big = [[I - LR*Wsym[IN:, IN:]],
                                          [   -LR*Wsym[:IN, IN:]]]

Kernel design (tensor parallel over 8 cores on one TRN2 chip):
  - M_big (bf16) is column-sharded: 1152 columns per core, SBUF-resident
    for all 500 steps (the only way to beat streaming 340MB/step from HBM).
  - The state r is kept TRANSPOSED and replicated: 72 k-tiles [128,16]
    (+ 4 constant x^T tiles).  Each step, each core:
      * 76 k-tiles x 4 column-group matmuls (tile_position col tiling:
        batch=16 occupies 4 x 16 PE columns), accumulating [16@32g, 288]
        in PSUM fp32;
      * ACT tanh -> bf16;  PE transposes (32-col blocks) back to k-tile
        layout;  DVE copy to a send buffer;
      * gpsimd remote_dma_broadcast sends its 9 tiles to all 8 cores'
        replicated state buffer (slot = own core id, double-buffered by
        step parity); monotonic semaphores with register thresholds
        synchronize across cores - no ncfw collectives, no loop barriers.
"""
from contextlib import ExitStack

import numpy as np
import ml_dtypes

IN_SZ = 512
TOTAL = 9216
REST = TOTAL - IN_SZ          # 8704
B = 16
LR = 0.5
N_CORES = 8
TPC = 9                        # state k-tiles per core
NX = 4                         # x k-tiles (4*128 = 512 = IN_SZ)
C = 128 * TPC                  # 1152 columns per core
GN = 32 * TPC                  # 288 columns per col group
NS = N_CORES * TPC             # 72 state k-tiles
KT = NS + NX                   # 76 contraction k-tiles
SL = 16 * TPC                  # 144 free elems per core slot (transposed)
RP = N_CORES * C               # 9216 padded rest size
NCH = 2                        # exchange chunks per step (pipelined sends)


def _ch_splits():
    """Split TPC tiles into NCH chunk boundaries, e.g. 9 -> [0,3,6,9]."""
    base, rem = divmod(TPC, NCH)
    sizes = [base + (1 if h < rem else 0) for h in range(NCH)]
    lo = [sum(sizes[:h]) for h in range(NCH)]
    return lo + [TPC]

NP_HALF = ml_dtypes.bfloat16


def _build_nc(T: int):
    import concourse.bass as bass
    import concourse.mybir as mybir
    from concourse import bacc

    F32 = mybir.dt.float32
    dt_half = mybir.dt.bfloat16
    assert T % 2 == 0 and T >= 4
    iters = T // 2

    nc = bacc.Bacc("TRN2", target_bir_lowering=False, debug=False,
                   num_devices=N_CORES)

    m_in = nc.dram_tensor("m_in", [128, KT * C], dt_half, kind="ExternalInput")
    xt_in = nc.dram_tensor("xt_in", [128, NX * 16], dt_half,
                           kind="ExternalInput")
    eye_in = nc.dram_tensor("eye_in", [128, 16], dt_half, kind="ExternalInput")
    out_d = nc.dram_tensor("out", [128, GN], F32, kind="ExternalOutput")

    m_sb = nc.alloc_sbuf_tensor("m_sb", [128, KT * C], dt_half)
    xt_sb = nc.alloc_sbuf_tensor("xt_sb", [128, NX * 16], dt_half)
    eye_sb = nc.alloc_sbuf_tensor("eye_sb", [128, 16], dt_half)
    recv_sb = nc.alloc_sbuf_tensor("recv_sb", [128, 2 * NS * 16], dt_half)
    send_sb = nc.alloc_sbuf_tensor("send_sb", [128, 2 * SL], dt_half)
    st_sb = nc.alloc_sbuf_tensor("st_sb", [128, 2 * GN], dt_half)
    out_sb = nc.alloc_sbuf_tensor("out_sb", [128, GN], F32)

    ps = [nc.alloc_psum_tensor(f"ps{k}", [128, GN], F32) for k in range(2)]
    pst = [nc.alloc_psum_tensor(f"pst{k}", [128, SL], dt_half)
           for k in range(2)]
    ps_scratch = nc.alloc_psum_tensor("ps_scratch", [128, GN], F32)

    ctx = ExitStack()
    nc._sem_ctx = ctx
    sem = lambda name: ctx.enter_context(nc.semaphore(name))
    s_load = sem("s_load")
    s_init = sem("s_init")
    # chunked exchange: chunk h = tiles [CH[h], CH[h+1])
    s_arr = [[sem(f"s_arr{h}_{k}") for k in range(2)] for h in range(NCH)]
    s_mm = [sem(f"s_mm{k}") for k in range(2)]
    s_tanh = [[sem(f"s_tanh{h}_{k}") for k in range(2)] for h in range(NCH)]
    s_tp = [[sem(f"s_tp{h}_{k}") for k in range(2)] for h in range(NCH)]
    s_cp = [[sem(f"s_cp{h}_{k}") for k in range(2)] for h in range(NCH)]
    s_prep = sem("s_prep")
    s_loc = sem("s_loc")
    s_out = sem("s_out")
    s_bar = sem("s_bar")

    n_dma_loads = KT + 2
    CH = _ch_splits()
    CH_LO = CH[:-1]
    CH_HI = CH[1:]
    CH_N = [CH_HI[h] - CH_LO[h] for h in range(NCH)]
    # k-loop consumption order: chunk-major (all cores' chunk-0 tiles first)
    K_ORDER = [(c, b) for h in range(NCH)
               for c in range(N_CORES) for b in range(CH_LO[h], CH_HI[h])]
    J0 = [N_CORES * CH_LO[h] for h in range(NCH)]

    def recv_ap(par, c, b):
        off = par * NS * 16 + c * SL + b * 16
        return recv_sb[:, off:off + 16]

    with nc.Block() as block:

        @block.sync
        def _(sync):
            for t in range(KT):
                sync.dma_start(m_sb[:, t * C:(t + 1) * C],
                               m_in[:, t * C:(t + 1) * C]).then_inc(s_load, 16)
            sync.dma_start(xt_sb[:, :], xt_in[:, :]).then_inc(s_load, 16)
            sync.dma_start(eye_sb[:, :], eye_in[:, :]).then_inc(s_load, 16)
            sync.wait_ge(s_out, 1)
            for h in range(NCH):
                for k in range(2):
                    sync.wait_ge(s_arr[h][k], 16 * (T // 2))
            sync.dma_start(out_d[:, :], out_sb[:, :]).then_inc(s_load, 16)
            sync.wait_ge(s_load, 16 * (n_dma_loads + 1))

        @block.vector
        def _(vector):
            vector.memset(recv_sb[:, :], 0).then_inc(s_init, 1)
            for k in range(2):
                vector.memset(ps[k][:, :], 0).then_inc(s_init, 1)
            with vector.Fori(0, iters) as i:
                for k in range(2):
                    # sends of steps <= s-1 drained (s_loc: +16 barrier,
                    # +16*NCH/step), then copy each chunk as it is transposed
                    vector.wait_ge(s_loc, ((i * 2 + k) * NCH + 1) * 16)
                    for h in range(NCH):
                        vector.wait_ge(s_tp[h][k], i + 1)
                        lo, hi = 16 * CH_LO[h], 16 * CH_HI[h]
                        vector.tensor_copy(
                            send_sb[:, k * SL + lo: k * SL + hi],
                            pst[k][:, lo:hi]).then_inc(s_cp[h][k], 1)

        @block.tensor
        def _(tensor):
            tensor.wait_ge(s_load, 16 * n_dma_loads)
            tensor.wait_ge(s_init, 3)

            N_DUMMY = 18

            def mm4(t, w, start, stop):
                mm = None
                for g in range(4):
                    mm = tensor.matmul(
                        ps[k_cur][32 * g:32 * g + B, :],
                        w,
                        m_sb[:, t * C + g * GN: t * C + (g + 1) * GN],
                        start=start, stop=stop,
                        tile_position=(0, 32 * g),
                        skip_group_check=True,
                    )
                return mm

            def step(i, k):
                nonlocal k_cur
                k_cur = k
                # constant x-tile matmuls first: real PE work that does not
                # depend on the exchange (keeps HAM warm during the wait)
                tensor.wait_ge(s_tanh[NCH - 1][k], i)  # psum WAR, tanh of s-2
                for t in range(NS, KT):
                    mm4(t, xt_sb[:, (t - NS) * 16:(t - NS) * 16 + 16],
                        t == NS, False)
                # chunk-0 tiles of every core arrive first; consume them
                # while later chunks' broadcasts are still in flight
                mm = None
                for h in range(NCH):
                    tensor.wait_ge(s_arr[h][k], (i + k) * 16)
                    for j in range(J0[h], J0[h] + N_CORES * CH_N[h]):
                        c, b = K_ORDER[j]
                        mm = mm4(j, recv_ap(k, c, b), False, j == NS - 1)
                mm.then_inc(s_mm[k], 1)
                # transposes + sends per chunk: early chunks' broadcasts
                # launch while later chunks are still being transposed
                for h in range(NCH):
                    tensor.wait_ge(s_tanh[h][k], i + 1)
                    tensor.wait_ge(s_cp[h][k], i)
                    tp = None
                    for b in range(CH_LO[h], CH_HI[h]):
                        for gi in range(4):
                            tp = tensor.transpose(
                                pst[k][32 * gi:32 * gi + 32,
                                       b * 16:(b + 1) * 16],
                                st_sb[32 * gi:32 * gi + B,
                                      k * GN + b * 32: k * GN + (b + 1) * 32],
                                eye_sb[32 * gi:32 * gi + B, :],
                                tile_position=(32 * gi, 32 * gi),
                            )
                    tp.then_inc(s_tp[h][k], 1)
                # dummy matmuls into a scratch bank: keep the PE array busy
                # while the broadcasts are in flight so HAM stays warm
                for dmy in range(N_DUMMY):
                    for g in range(4):
                        tensor.matmul(
                            ps_scratch[32 * g:32 * g + B, :],
                            xt_sb[:, 0:16],
                            m_sb[:, g * GN:(g + 1) * GN],
                            start=True, stop=True,
                            tile_position=(0, 32 * g),
                            skip_group_check=True,
                        )

            k_cur = 0
            with tensor.Fori(0, iters) as i:
                step(i, 0)
                step(i, 1)

        @block.scalar
        def _(scalar):
            import concourse.mybir as mybir
            with scalar.Fori(0, iters) as i:
                for k in range(2):
                    scalar.wait_ge(s_mm[k], i + 1)
                    for h in range(NCH):
                        scalar.wait_ge(s_tp[h][k], i)   # WAR on st_sb
                        lo, hi = 32 * CH_LO[h], 32 * CH_HI[h]
                        scalar.activation(
                            st_sb[:, k * GN + lo: k * GN + hi],
                            ps[k][:, lo:hi],
                            mybir.ActivationFunctionType.Tanh,
                        ).then_inc(s_tanh[h][k], 1)
            kf = (T - 1) % 2
            scalar.wait_ge(s_mm[kf], iters)
            scalar.activation(out_sb[:, :], ps[kf][:, :],
                              mybir.ActivationFunctionType.Tanh,
                              ).then_inc(s_out, 1)

        @block.gpsimd
        def _(gpsimd):
            pid = gpsimd.partition_id()
            rdests = [(0, kk) for kk in range(N_CORES)]
            gpsimd.wait_ge(s_init, 3)
            gpsimd.remote_sem_update_broadcast(
                s_bar, s_loc, rdests=rdests).then_inc(s_prep, 1)
            gpsimd.wait_ge(s_prep, 1)
            gpsimd.trigger_dma(1)
            gpsimd.wait_ge(s_bar, 16)
            with gpsimd.Fori(0, iters) as i:
                for k in range(2):
                    dst_par = 1 - k
                    for h in range(NCH):
                        lo, hi = 16 * CH_LO[h], 16 * CH_HI[h]
                        out_ap = recv_sb[:, bass.ds(
                            pid * SL + dst_par * NS * 16 + lo, hi - lo)]
                        gpsimd.remote_dma_broadcast(
                            out_ap, send_sb[:, k * SL + lo: k * SL + hi],
                            remote_sem=s_arr[h][dst_par], local_sem=s_loc,
                            rdests=rdests,
                        ).then_inc(s_prep, 1)
                        gpsimd.wait_ge(s_cp[h][k], i + 1)
                        gpsimd.wait_ge(s_prep, (i * 2 + k) * NCH + h + 2)
                        gpsimd.trigger_dma(1)
            gpsimd.wait_ge(s_loc, 16 * (NCH * T + 1))
            for h in range(NCH):
                for k in range(2):
                    gpsimd.wait_ge(s_arr[h][k], 16 * (T // 2))

    nc.compile()
    return nc


def _prep_inputs(x: np.ndarray, W: np.ndarray):
    Wsym = 0.5 * (W.astype(np.float32) + W.astype(np.float32).T)
    M_big = np.zeros((KT * 128, RP), dtype=np.float32)
    M_big[:REST, :REST] = -LR * Wsym[IN_SZ:, IN_SZ:]
    M_big[:REST, :REST] += np.eye(REST, dtype=np.float32)
    M_big[RP:RP + IN_SZ, :REST] = -LR * Wsym[:IN_SZ, IN_SZ:]

    perm = np.empty(C, dtype=np.int64)
    p = 0
    for g in range(4):
        for j in range(TPC):
            perm[p:p + 32] = 128 * j + 32 * g + np.arange(32)
            p += 32

    xt = np.zeros((128, NX * 16), dtype=np.float32)
    for u in range(NX):
        xt[:, u * 16:u * 16 + B] = x[:, 128 * u:128 * (u + 1)].T
    eye = np.zeros((128, 16), dtype=np.float32)
    for gi in range(4):
        eye[32 * gi + np.arange(16), np.arange(16)] = 1.0

    xt_h = np.ascontiguousarray(xt.astype(NP_HALF))
    eye_h = np.ascontiguousarray(eye.astype(NP_HALF))
    # k-loop block order (must match K_ORDER in _build_nc): chunk-major
    CH = _ch_splits()
    order = ([TPC * cc + b for h in range(NCH)
              for cc in range(N_CORES) for b in range(CH[h], CH[h + 1])]
             + list(range(NS, KT)))          # x blocks stay at the end
    in_maps = []
    for c in range(N_CORES):
        shard = M_big[:, c * C:(c + 1) * C][:, perm]
        m_loc = shard.reshape(KT, 128, C)[order]
        m_loc = m_loc.transpose(1, 0, 2).reshape(128, -1)
        in_maps.append({
            "m_in": np.ascontiguousarray(m_loc.astype(NP_HALF)),
            "xt_in": xt_h,
            "eye_in": eye_h,
        })
    return in_maps


def _extract_output(out_core: np.ndarray):
    """out_core: [128, GN] fp32 from core 7; output = global cols 8192:8704
    = core 7 local cols 128:640 (pre-permutation)."""
    res = np.empty((B, 512), dtype=np.float32)
    for cl in range(128, 640):
        m = cl // 32
        res[:, cl - 128] = out_core[32 * (m % 4):32 * (m % 4) + B,
                                    32 * (m // 4) + (cl % 32)]
    return res


_CACHE = {}


def _make_exec(T):
    """Compile the NEFF and build a jitted shard_map executor whose inputs
    are device-resident jax Arrays — avoids the ~180MB host->device upload
    landing inside the NEFF execution window (which skews core start times
    by several ms and stalls the first cross-core exchange)."""
    import jax
    import numpy as np
    from jax.experimental.shard_map import shard_map
    from jax.sharding import Mesh, PartitionSpec
    import concourse.mybir as mybir
    from concourse import bass2jax

    bass2jax.install_neuronx_cc_hook()
    nc = _build_nc(T)

    partition_name = (nc.partition_id_tensor.name
                      if nc.partition_id_tensor else None)
    in_names, out_names, out_avals, zero_shapes = [], [], [], []
    for alloc in nc.m.functions[0].allocations:
        if not isinstance(alloc, mybir.MemoryLocationSet):
            continue
        name = alloc.memorylocations[0].name
        if alloc.kind == "ExternalInput":
            if name != partition_name:
                in_names.append(name)
        elif alloc.kind == "ExternalOutput":
            shape = tuple(alloc.tensor_shape)
            dtype = mybir.dt.np(alloc.dtype)
            out_names.append(name)
            out_avals.append(jax.core.ShapedArray(shape, dtype))
            zero_shapes.append((shape, dtype))
    n_params = len(in_names)
    all_names = list(in_names) + list(out_names)
    if partition_name is not None:
        all_names.append(partition_name)
    donate = tuple(range(n_params, n_params + len(out_names)))

    perdev = bool(int(__import__("os").environ.get("KERNEL_PERDEV", "0")))

    if perdev:
        # 8 independent single-device executions dispatched asynchronously,
        # with the partition id fed as an explicit input (PartitionIdOp only
        # yields the right rank under shard_map).
        def _body(*args):
            outs = bass2jax._bass_exec_p.bind(
                *args,
                out_avals=tuple(out_avals),
                in_names=tuple(all_names),
                out_names=tuple(out_names),
                lowering_input_output_aliases=(),
                sim_require_finite=True,
                sim_require_nnan=True,
                nc=nc,
            )
            return tuple(outs)

        devices = jax.devices()[:N_CORES]
        fn = jax.jit(_body, donate_argnums=donate, keep_unused=True)
        return dict(nc=nc, fn=fn, in_names=in_names, out_names=out_names,
                    zero_shapes=zero_shapes, devices=devices,
                    partition_name=partition_name, perdev=True)

    def _body(*args):
        operands = list(args)
        if partition_name is not None:
            operands.append(bass2jax.partition_id_tensor())
        outs = bass2jax._bass_exec_p.bind(
            *operands,
            out_avals=tuple(out_avals),
            in_names=tuple(all_names),
            out_names=tuple(out_names),
            lowering_input_output_aliases=(),
            sim_require_finite=True,
            sim_require_nnan=True,
            nc=nc,
        )
        return tuple(outs)

    devices = jax.devices()[:N_CORES]
    mesh = Mesh(np.asarray(devices), ("core",))
    nspecs = n_params + len(out_names)
    sharded = jax.jit(
        shard_map(_body, mesh=mesh,
                  in_specs=(PartitionSpec("core"),) * nspecs,
                  out_specs=(PartitionSpec("core"),) * len(out_names),
                  check_rep=False),
        donate_argnums=donate, keep_unused=True,
    )
    shard = jax.sharding.NamedSharding(mesh, PartitionSpec("core"))
    return dict(nc=nc, fn=sharded, in_names=in_names, out_names=out_names,
                zero_shapes=zero_shapes, shard=shard, perdev=False)


def _run(x, W, T, trace=False):
    """Returns {core_id: {out_name: np.ndarray}} like BassKernelResults.results
    plus None exec time (use test.py for profiling)."""
    import jax
    if T not in _CACHE:
        _CACHE[T] = _make_exec(T)
    ex = _CACHE[T]
    in_maps = _prep_inputs(x, W)
    import numpy as np

    class _Res:
        pass

    res = _Res()
    res.exec_time_ns = None

    if ex.get("perdev"):
        per_core_outs = _run_perdev(ex, in_maps)
        res.results = per_core_outs
        return res

    placed = [
        jax.device_put(
            np.concatenate([in_maps[c][name] for c in range(N_CORES)], axis=0),
            ex["shard"])
        for name in ex["in_names"]
    ]
    zeros = [
        jax.device_put(np.zeros((N_CORES * s[0], *s[1:]), dt), ex["shard"])
        for (s, dt) in ex["zero_shapes"]
    ]
    for a in placed + zeros:
        a.block_until_ready()
    outs = ex["fn"](*placed, *zeros)
    outs = [np.asarray(o) for o in outs]
    res.results = [
        {name: outs[i].reshape(N_CORES, -1, *outs[i].shape[1:])[c]
         for i, name in enumerate(ex["out_names"])}
        for c in range(N_CORES)
    ]
    return res


def _run_perdev(ex, in_maps):
    import jax
    import numpy as np
    devices = ex["devices"]
    all_args = []
    for c in range(N_CORES):
        args = [jax.device_put(in_maps[c][n], devices[c])
                for n in ex["in_names"]]
        args += [jax.device_put(np.zeros(s, dt), devices[c])
                 for (s, dt) in ex["zero_shapes"]]
        if ex["partition_name"] is not None:
            args.append(jax.device_put(np.array([[c]], np.uint32),
                                       devices[c]))
        all_args.append(args)
    for args in all_args:
        for a in args:
            a.block_until_ready()
    futs = [ex["fn"](*all_args[c]) for c in range(N_CORES)]  # async dispatch
    outs = [[np.asarray(o) for o in f] for f in futs]
    return [
        {name: outs[c][i] for i, name in enumerate(ex["out_names"])}
        for c in range(N_CORES)
    ]


def kernel(x: np.ndarray, W: np.ndarray, T_settle) -> np.ndarray:
    T = int(np.asarray(T_settle))
    res = _run(np.asarray(x, np.float32), np.asarray(W, np.float32), T)
    return _extract_output(res.results[7]["out"])
